# revision 29
# baseline (speedup 1.0000x reference)
"""Trainium2 Bass kernel for nn_ChimeraV2Block (dual-softmax differential
sliding-window attention block, B=1 S=2048 D=2048, 16 q-heads / 4 kv-heads,
head_dim 128, window 512).

Sharding: tensor-parallel over heads across 8 NeuronCores. Core c owns
q-heads {2c, 2c+1} and kv-head c//2 (GQA groups align with the split).
Wq/Wk/Wv column-sharded, Wo row-sharded; the 8 fp32 partial outputs are
summed on the host (the "all-reduce").

Schedule: phase 1 (projections + RoPE) is PE-bound; phase 2/3 (attention +
output projection) is software-pipelined two q-tiles deep so the per-tile
cross-engine chain (exp -> recip -> g0 -> relu -> gn) overlaps the next
tile's score matmuls, with the Wo matmuls and output DMA interleaved
per-tile. Masks are applied only to the two boundary 128-col subtiles of
each 640-wide score strip.
"""

import sys

if "/opt/trn_rl_repo" not in sys.path:
    sys.path.insert(0, "/opt/trn_rl_repo")

import numpy as np
import ml_dtypes

BF = ml_dtypes.bfloat16
F16 = np.float16

S = 2048
D = 2048
H = 16
HK = 4
HD = 128
WIN = 512
THETA = 10000.0
N_CORES = 8
NQT = S // 128          # 16 q row-tiles
NKT = D // 128          # 16 contraction tiles for the projections
WMAX = WIN + 128        # 640: max key-window width per q-tile
NEG = -1.0e30

_CACHE = {}


def _tables():
    """RoPE tables [128, S] fp16 with head-dim-duplicated frequencies
    (row p uses invf[p % 64]), so every op reads the table at the same
    base partition as its (possibly swapped) q operand. Q tables are
    pre-scaled by the attention scale 1/sqrt(64)."""
    invf = 1.0 / (THETA ** (np.arange(0, HD, 2, dtype=np.float64) / HD))  # [64]
    t = np.arange(S, dtype=np.float64)
    fr = np.outer(invf, t)  # [64, S]
    cosf = np.concatenate([np.cos(fr)] * 2, axis=0)
    sinf = np.concatenate([np.sin(fr)] * 2, axis=0)
    return (np.ascontiguousarray(cosf * 0.125, dtype=F16),
            np.ascontiguousarray(sinf * 0.125, dtype=F16),
            np.ascontiguousarray(cosf, dtype=F16),
            np.ascontiguousarray(sinf, dtype=F16))


def _masks():
    p = np.arange(128)[:, None]
    c = np.arange(128)[None, :]
    # interior first subtile: key col c (window start) allowed iff c >= p+1
    m_lo = np.where(c >= p + 1, 0.0, NEG).astype(BF)
    # causal subtile (diagonal block): allowed iff c <= p
    mc = np.where(c <= p, 0.0, NEG).astype(BF)
    return m_lo, mc


def _build_program():
    import concourse.bacc as bacc
    import concourse.tile as tile
    from concourse import mybir

    bf = mybir.dt.bfloat16
    f16 = mybir.dt.float16
    f32 = mybir.dt.float32
    EXP = mybir.ActivationFunctionType.Exp
    MULT = mybir.AluOpType.mult
    ADD = mybir.AluOpType.add
    MAX = mybir.AluOpType.max
    DIV = mybir.AluOpType.divide

    nc = bacc.Bacc("TRN2", target_bir_lowering=False, debug=False,
                   num_devices=N_CORES)

    xt_d = nc.dram_tensor("xt", [128, NKT, S], bf, kind="ExternalInput")
    wq_d = nc.dram_tensor("wq", [128, NKT, 2, 128], bf, kind="ExternalInput")
    wk_d = nc.dram_tensor("wk", [128, NKT, 128], bf, kind="ExternalInput")
    wv_d = nc.dram_tensor("wv", [128, NKT, 128], bf, kind="ExternalInput")
    wo_d = nc.dram_tensor("wo", [128, 2, D], bf, kind="ExternalInput")
    lamn_d = nc.dram_tensor("lamn", [1, 2], f32, kind="ExternalInput")
    out_d = nc.dram_tensor("outp", [S, D], f16, kind="ExternalOutput")

    tqc_np, tqs_np, tkc_np, tks_np = _tables()
    mlo_np, mc_np = _masks()
    tqc_d = nc.inline_tensor(tqc_np, "tab_qc")
    tqs_d = nc.inline_tensor(tqs_np, "tab_qs")
    tkc_d = nc.inline_tensor(tkc_np, "tab_kc")
    tks_d = nc.inline_tensor(tks_np, "tab_ks")
    mlo_d = nc.inline_tensor(mlo_np, "mask_lo")
    mc_d = nc.inline_tensor(mc_np, "mask_c")
    idb_d = nc.inline_tensor(np.eye(128, dtype=BF), "ident_bf")
    ones_d = nc.inline_tensor(np.ones((128, 1), dtype=np.float32), "ones_f32")

    with tile.TileContext(nc) as tc:
        with tc.tile_pool(name="xpool", bufs=1) as xp, \
             tc.tile_pool(name="wpool", bufs=1) as wp, \
             tc.tile_pool(name="pers", bufs=1) as pers:

            # ---- input DMAs, critical-path first: interleave the weight
            # slices and first x chunk so every queue carries bytes the
            # first projection matmuls need.
            wq = wp.tile([128, NKT, 2, 128], bf)
            wk = wp.tile([128, NKT, 128], bf)
            wv = wp.tile([128, NKT, 128], bf)
            xt = xp.tile([128, NKT, S], bf)
            nc.sync.dma_start(out=wq[:, 0:1], in_=wq_d[:, 0:1])
            nc.sync.dma_start(out=wk[:, 0:2], in_=wk_d[:, 0:2])
            nc.sync.dma_start(out=wv[:, 0:2], in_=wv_d[:, 0:2])
            nc.sync.dma_start(out=xt[:, 0:1, 0:512], in_=xt_d[:, 0:1, 0:512])
            nc.sync.dma_start(out=xt[:, 1:2, 0:512], in_=xt_d[:, 1:2, 0:512])
            nc.sync.dma_start(out=wq[:, 1:3], in_=wq_d[:, 1:3])
            nc.sync.dma_start(out=xt[:, 2:3, 0:512], in_=xt_d[:, 2:3, 0:512])
            nc.sync.dma_start(out=xt[:, 3:4, 0:512], in_=xt_d[:, 3:4, 0:512])
            nc.sync.dma_start(out=wq[:, 3:6], in_=wq_d[:, 3:6])
            nc.sync.dma_start(out=wk[:, 2:8], in_=wk_d[:, 2:8])
            nc.sync.dma_start(out=wv[:, 2:8], in_=wv_d[:, 2:8])
            nc.sync.dma_start(out=xt[:, 4:6, 0:512], in_=xt_d[:, 4:6, 0:512])
            nc.sync.dma_start(out=xt[:, 6:8, 0:512], in_=xt_d[:, 6:8, 0:512])
            nc.sync.dma_start(out=wq[:, 6:11], in_=wq_d[:, 6:11])
            nc.sync.dma_start(out=wk[:, 8:16], in_=wk_d[:, 8:16])
            nc.sync.dma_start(out=wv[:, 8:16], in_=wv_d[:, 8:16])
            nc.sync.dma_start(out=xt[:, 8:12, 0:512], in_=xt_d[:, 8:12, 0:512])
            nc.sync.dma_start(out=wq[:, 11:16], in_=wq_d[:, 11:16])
            nc.sync.dma_start(out=xt[:, 12:16, 0:512], in_=xt_d[:, 12:16, 0:512])
            tqc = wp.tile([128, S], f16)
            tqs = wp.tile([128, S], f16)
            tkc = wp.tile([128, S], f16)
            tks = wp.tile([128, S], f16)
            nc.sync.dma_start(out=tqc[:], in_=tqc_d[:])
            nc.sync.dma_start(out=tqs[:], in_=tqs_d[:])
            nc.sync.dma_start(out=tkc[:], in_=tkc_d[:])
            nc.sync.dma_start(out=tks[:], in_=tks_d[:])
            mlo = wp.tile([128, 128], bf)
            nc.sync.dma_start(out=mlo[:], in_=mlo_d[:])
            mcs = wp.tile([128, 128], bf)
            nc.sync.dma_start(out=mcs[:], in_=mc_d[:])
            idb = wp.tile([128, 128], bf)
            nc.sync.dma_start(out=idb[:], in_=idb_d[:])
            lamn = wp.tile([1, 2], f32)
            nc.sync.dma_start(out=lamn[:], in_=lamn_d[:])
            ones = wp.tile([128, 1], f32)
            nc.sync.dma_start(out=ones[:], in_=ones_d[:])
            for ch in range(1, 4):
                sl = slice(512 * ch, 512 * (ch + 1))
                for kq in range(4):
                    ks = slice(4 * kq, 4 * kq + 4)
                    nc.sync.dma_start(out=xt[:, ks, sl], in_=xt_d[:, ks, sl])
            wo = wp.tile([128, 2, D], bf)
            nc.sync.dma_start(out=wo[:, :, 0:1024], in_=wo_d[:, :, 0:1024])
            nc.sync.dma_start(out=wo[:, :, 1024:2048], in_=wo_d[:, :, 1024:2048])
            lamb = wp.tile([128, 2], f32)
            nc.gpsimd.partition_broadcast(lamb[:], lamn[:])

            qt = pers.tile([128, 2, S], f16)      # RoPE'd scaled q, hd-major
            kt = pers.tile([128, S], f16)         # RoPE'd k, hd-major
            vsm = pers.tile([128, NQT, 128], bf)  # v, S-major [s, hd]
            att = pers.tile([128, 2, S], bf)      # attention out^T, hd-major

            # ---- Phase 1: projections + RoPE + v transpose ----
            with tc.tile_pool(name="pp", bufs=1, space="PSUM") as pp, \
                 tc.tile_pool(name="pt", bufs=2) as pt:
                for nch in range(4):
                    sl = slice(nch * 512, (nch + 1) * 512)
                    ps_q0 = pp.tile([128, 512], f32, tag="pq0", bufs=2)
                    ps_q1 = pp.tile([128, 512], f32, tag="pq1", bufs=2)
                    ps_k = pp.tile([128, 512], f32, tag="pk", bufs=1)
                    ps_v = pp.tile([128, 512], f32, tag="pv", bufs=1)
                    for kti in range(NKT):
                        st = kti == 0
                        sp = kti == NKT - 1
                        rhs = xt[:, kti, sl]
                        nc.tensor.matmul(ps_q0[:], wq[:, kti, 0, :], rhs, start=st, stop=sp)
                        nc.tensor.matmul(ps_q1[:], wq[:, kti, 1, :], rhs, start=st, stop=sp)
                        nc.tensor.matmul(ps_k[:], wk[:, kti, :], rhs, start=st, stop=sp)
                        nc.tensor.matmul(ps_v[:], wv[:, kti, :], rhs, start=st, stop=sp)
                    for ps, outt, tabc, tabs in (
                            (ps_q0, qt[:, 0, sl], tqc, tqs),
                            (ps_q1, qt[:, 1, sl], tqc, tqs),
                            (ps_k, kt[:, sl], tkc, tks)):
                        f = pt.tile([128, 512], f16, tag="f", bufs=4)
                        nc.scalar.copy(out=f[:], in_=ps[:])
                        m2 = pt.tile([128, 512], f16, tag="m2")
                        # rotate_half partner * sin
                        nc.vector.tensor_mul(m2[0:64, :], f[64:128, :], tabs[64:128, sl])
                        nc.vector.tensor_mul(m2[64:128, :], f[0:64, :], tabs[0:64, sl])
                        m1 = pt.tile([128, 512], f16, tag="m1")
                        nc.vector.tensor_mul(m1[:], f[:], tabc[:, sl])
                        nc.vector.tensor_sub(outt[0:64, :], m1[0:64, :], m2[0:64, :])
                        nc.vector.tensor_add(outt[64:128, :], m1[64:128, :], m2[64:128, :])
                    vtmp = pt.tile([128, 512], bf, tag="vtmp")
                    nc.scalar.copy(out=vtmp[:], in_=ps_v[:])
                    ps_tv = pp.tile([128, 4, 128], bf, tag="ptv", bufs=2)
                    for j in range(4):
                        nc.tensor.transpose(ps_tv[:, j, :], vtmp[:, 128 * j:128 * (j + 1)], idb[:])
                    nc.vector.tensor_copy(out=vsm[:, 4 * nch:4 * (nch + 1), :], in_=ps_tv[:])

            # ---- Phase 2+3: attention with 2-deep pipeline + Wo ----
            with tc.tile_pool(name="psc", bufs=1, space="PSUM") as psc, \
                 tc.tile_pool(name="pse", bufs=1) as pse, \
                 tc.tile_pool(name="psm", bufs=1) as psm:

                gn_t = {}

                def emit_scores_h(qi, h):
                    qsl = slice(qi * 128, (qi + 1) * 128)
                    kw = min(qi + 1, 5)
                    w = kw * 128
                    kstart = max(0, qi - 4)
                    if True:
                        ps_s = [psc.tile([128, WMAX], f32, tag="s", bufs=2,
                                         name=f"ps_s{half}")
                                for half in range(2)]
                        for half, ps in enumerate(ps_s):
                            hp = slice(64 * half, 64 * half + 64)
                            lhs = qt[hp, h, qsl]
                            ktw = kt[hp, kstart * 128:kstart * 128 + w]
                            if qi >= 4:
                                nc.tensor.matmul(ps[:, 0:128], idb[:], mlo[:],
                                                 start=True, stop=False)
                                nc.tensor.matmul(ps[:, 0:128], lhs, ktw[:, 0:128],
                                                 start=False, stop=True)
                                nc.tensor.matmul(ps[:, 128:512], lhs,
                                                 ktw[:, 128:512], start=True, stop=True)
                                nc.tensor.matmul(ps[:, 512:640], idb[:], mcs[:],
                                                 start=True, stop=False)
                                nc.tensor.matmul(ps[:, 512:640], lhs,
                                                 ktw[:, 512:640], start=False, stop=True)
                            else:
                                wu = (kw - 1) * 128
                                if wu > 0:
                                    nc.tensor.matmul(ps[:, 0:wu], lhs, ktw[:, 0:wu],
                                                     start=True, stop=True)
                                nc.tensor.matmul(ps[:, wu:w], idb[:], mcs[:],
                                                 start=True, stop=False)
                                nc.tensor.matmul(ps[:, wu:w], lhs, ktw[:, wu:w],
                                                 start=False, stop=True)

                        e1 = pse.tile([128, WMAX], bf, tag="e1", bufs=2)
                        e2 = pse.tile([128, WMAX], bf, tag="e2", bufs=2)
                        s1 = psm.tile([128, 1], f32, tag="s1", bufs=4)
                        s2 = psm.tile([128, 1], f32, tag="s2", bufs=4)
                        # e2 first so r2's reciprocal overlaps e1's exp
                        nc.scalar.activation(out=e2[:, 0:w], in_=ps_s[1][:, 0:w],
                                             func=EXP, accum_out=s2[:])
                        nc.scalar.activation(out=e1[:, 0:w], in_=ps_s[0][:, 0:w],
                                             func=EXP, accum_out=s1[:])
                        # cneg = -(lam * s1 / s2)   (lamn holds -lam)
                        r2 = psm.tile([128, 1], f32, tag="r2", bufs=4)
                        nc.vector.reciprocal(out=r2[:], in_=s2[:])
                        cneg = psm.tile([128, 1], f32, tag="cneg", bufs=4)
                        nc.gpsimd.tensor_scalar(
                            out=cneg[:], in0=s1[:], scalar1=lamb[:, h:h + 1],
                            scalar2=r2[:], op0=MULT, op1=MULT)
                        # g = relu(e1 + cneg*e2), normalized by its row sum
                        g0 = pse.tile([128, WMAX], bf, tag="g0", bufs=2)
                        nc.vector.scalar_tensor_tensor(
                            out=g0[:, 0:w], in0=e2[:, 0:w], scalar=cneg[:],
                            in1=e1[:, 0:w], op0=MULT, op1=ADD)
                        g = pse.tile([128, WMAX], bf, tag="g", bufs=2)
                        dsum = psm.tile([128, 1], f32, tag="dsum", bufs=4)
                        nc.vector.tensor_scalar(
                            out=g[:, 0:w], in0=g0[:, 0:w], scalar1=0.0,
                            scalar2=0.0, op0=MAX, op1=ADD, accum_out=dsum[:])
                        recd = psm.tile([128, 1], f32, tag="recd", bufs=4)
                        nc.vector.reciprocal(out=recd[:], in_=dsum[:])
                        gn = pse.tile([128, WMAX], bf, tag="gn", bufs=4)
                        nc.gpsimd.tensor_scalar(
                            out=gn[:, 0:w], in0=g[:, 0:w], scalar1=recd[:],
                            scalar2=0.0, op0=MULT, op1=ADD)
                        gn_t[(qi, h)] = gn

                def emit_ph3_pair(qj, pair):
                    qsl = slice(qj * 128, (qj + 1) * 128)
                    for dch in (2 * pair, 2 * pair + 1):
                        dsl = slice(dch * 512, (dch + 1) * 512)
                        ps_o = psc.tile([128, 512], f32, tag="o", bufs=2)
                        nc.tensor.matmul(ps_o[:], att[:, 0, qsl], wo[:, 0, dsl],
                                         start=True, stop=False)
                        nc.tensor.matmul(ps_o[:], att[:, 1, qsl], wo[:, 1, dsl],
                                         start=False, stop=True)
                        so = pse.tile([128, 512], f16, tag="so", bufs=4)
                        if dch % 2 == 0:
                            nc.scalar.copy(out=so[:], in_=ps_o[:])
                        else:
                            nc.vector.tensor_copy(out=so[:], in_=ps_o[:])
                        nc.sync.dma_start(out=out_d[qsl, dsl], in_=so[:])

                gts_t = {}

                def emit_transp(qi, h):
                    kw = min(qi + 1, 5)
                    if h == 0:
                        gts_t[qi] = pse.tile([128, 5, 2, 128], bf, tag="gts",
                                             bufs=2, name="gts2")
                    gts2 = gts_t[qi]
                    gn = gn_t.pop((qi, h))
                    ps_tr = psc.tile([128, 5, 128], bf, tag="trg", bufs=1)
                    for j in range(kw):
                        nc.tensor.transpose(ps_tr[:, j, :],
                                            gn[:, 128 * j:128 * (j + 1)], idb[:])
                    if h == 0:
                        nc.scalar.copy(out=gts2[:, 0:kw, h, :], in_=ps_tr[:, 0:kw, :])
                    else:
                        nc.vector.tensor_copy(out=gts2[:, 0:kw, h, :], in_=ps_tr[:, 0:kw, :])

                def emit_av(qi):
                    qsl = slice(qi * 128, (qi + 1) * 128)
                    kw = min(qi + 1, 5)
                    kstart = max(0, qi - 4)
                    ps_av = psc.tile([128, 2, 128], f32, tag="av", bufs=1)
                    gts2 = gts_t.pop(qi)
                    for j in range(kw):
                        nc.tensor.matmul(ps_av[:, :, :], vsm[:, kstart + j, :],
                                         gts2[:, j, :, :],
                                         start=(j == 0), stop=(j == kw - 1))
                    nc.vector.tensor_copy(out=att[:, :, qsl], in_=ps_av[:])

                # Per iteration i the PE queue alternates independent work so
                # no copy/exp dependency ever leaves it idle:
                #   scores(i,h) | transposes(i-1,h) | Wo pair(i-2,h) | AV(i-1)
                for i in range(NQT + 2):
                    for h in range(2):
                        if i < NQT:
                            emit_scores_h(i, h)
                        if 0 <= i - 1 < NQT:
                            emit_transp(i - 1, h)
                        if 0 <= i - 2 < NQT:
                            emit_ph3_pair(i - 2, h)
                    if 0 <= i - 1 < NQT:
                        emit_av(i - 1)

    nc.compile()
    return nc


def get_program():
    if "nc" not in _CACHE:
        _CACHE["nc"] = _build_program()
    return _CACHE["nc"]


def _prep_inputs(x, Wq, Wk, Wv, Wo, lam):
    xt = np.ascontiguousarray(x.reshape(S, D).T.astype(BF)
                              .reshape(NKT, 128, S).transpose(1, 0, 2))
    in_maps = []
    for c in range(N_CORES):
        h0 = 2 * c
        kv = c // 2
        wq_c = np.ascontiguousarray(
            Wq[:, h0 * 128:(h0 + 2) * 128].astype(BF)
            .reshape(NKT, 128, 2, 128).transpose(1, 0, 2, 3))
        wk_c = np.ascontiguousarray(
            Wk[:, kv * 128:(kv + 1) * 128].astype(BF)
            .reshape(NKT, 128, 128).transpose(1, 0, 2))
        wv_c = np.ascontiguousarray(
            Wv[:, kv * 128:(kv + 1) * 128].astype(BF)
            .reshape(NKT, 128, 128).transpose(1, 0, 2))
        wo_c = np.ascontiguousarray(
            Wo[h0 * 128:(h0 + 2) * 128, :].astype(BF)
            .reshape(2, 128, D).transpose(1, 0, 2))
        lamn_c = np.array([[-float(lam[h0]), -float(lam[h0 + 1])]], dtype=np.float32)
        in_maps.append({"xt": xt, "wq": wq_c, "wk": wk_c, "wv": wv_c,
                        "wo": wo_c, "lamn": lamn_c})
    return in_maps


def kernel(x, Wq, Wk, Wv, Wo, lam):
    from concourse.bass_utils import run_bass_kernel_spmd

    nc = get_program()
    in_maps = _prep_inputs(np.asarray(x), np.asarray(Wq), np.asarray(Wk),
                           np.asarray(Wv), np.asarray(Wo), np.asarray(lam))
    res = run_bass_kernel_spmd(nc, in_maps, list(range(N_CORES)))
    out = np.zeros((S, D), dtype=np.float32)
    for c in range(N_CORES):
        out += res.results[c]["outp"].astype(np.float32)
    return out.reshape(1, S, D)


# revision 31
# speedup vs baseline: 1.0087x; 1.0087x over previous
"""Trainium2 Bass kernel for nn_ChimeraV2Block (dual-softmax differential
sliding-window attention block, B=1 S=2048 D=2048, 16 q-heads / 4 kv-heads,
head_dim 128, window 512).

Sharding: tensor-parallel over heads across 8 NeuronCores. Core c owns
q-heads {2c, 2c+1} and kv-head c//2 (GQA groups align with the split).
Wq/Wk/Wv column-sharded, Wo row-sharded; the 8 fp32 partial outputs are
summed on the host (the "all-reduce").

Schedule: phase 1 (projections + RoPE) is PE-bound; phase 2/3 (attention +
output projection) is software-pipelined two q-tiles deep so the per-tile
cross-engine chain (exp -> recip -> g0 -> relu -> gn) overlaps the next
tile's score matmuls, with the Wo matmuls and output DMA interleaved
per-tile. Masks are applied only to the two boundary 128-col subtiles of
each 640-wide score strip.
"""

import sys

if "/opt/trn_rl_repo" not in sys.path:
    sys.path.insert(0, "/opt/trn_rl_repo")

import numpy as np
import ml_dtypes

BF = ml_dtypes.bfloat16
F16 = np.float16

S = 2048
D = 2048
H = 16
HK = 4
HD = 128
WIN = 512
THETA = 10000.0
N_CORES = 8
NQT = S // 128          # 16 q row-tiles
NKT = D // 128          # 16 contraction tiles for the projections
WMAX = WIN + 128        # 640: max key-window width per q-tile
NEG = -1.0e30

_CACHE = {}


def _tables():
    """RoPE tables [128, S] fp16 with head-dim-duplicated frequencies
    (row p uses invf[p % 64]), so every op reads the table at the same
    base partition as its (possibly swapped) q operand. Q tables are
    pre-scaled by the attention scale 1/sqrt(64)."""
    invf = 1.0 / (THETA ** (np.arange(0, HD, 2, dtype=np.float64) / HD))  # [64]
    t = np.arange(S, dtype=np.float64)
    fr = np.outer(invf, t)  # [64, S]
    cosf = np.concatenate([np.cos(fr)] * 2, axis=0)
    sinf = np.concatenate([np.sin(fr)] * 2, axis=0)
    return (np.ascontiguousarray(cosf * 0.125, dtype=F16),
            np.ascontiguousarray(sinf * 0.125, dtype=F16),
            np.ascontiguousarray(cosf, dtype=F16),
            np.ascontiguousarray(sinf, dtype=F16))


def _masks():
    p = np.arange(128)[:, None]
    c = np.arange(128)[None, :]
    # interior first subtile: key col c (window start) allowed iff c >= p+1
    m_lo = np.where(c >= p + 1, 0.0, NEG).astype(BF)
    # causal subtile (diagonal block): allowed iff c <= p
    mc = np.where(c <= p, 0.0, NEG).astype(BF)
    return m_lo, mc


def _build_program():
    import concourse.bacc as bacc
    import concourse.tile as tile
    from concourse import mybir

    bf = mybir.dt.bfloat16
    f16 = mybir.dt.float16
    f32 = mybir.dt.float32
    EXP = mybir.ActivationFunctionType.Exp
    MULT = mybir.AluOpType.mult
    ADD = mybir.AluOpType.add
    MAX = mybir.AluOpType.max
    DIV = mybir.AluOpType.divide

    nc = bacc.Bacc("TRN2", target_bir_lowering=False, debug=False,
                   num_devices=N_CORES)

    xt_d = nc.dram_tensor("xt", [128, NKT, S], bf, kind="ExternalInput")
    wq_d = nc.dram_tensor("wq", [128, NKT, 2, 128], bf, kind="ExternalInput")
    wk_d = nc.dram_tensor("wk", [128, NKT, 128], bf, kind="ExternalInput")
    wv_d = nc.dram_tensor("wv", [128, NKT, 128], bf, kind="ExternalInput")
    wo_d = nc.dram_tensor("wo", [128, 2, D], bf, kind="ExternalInput")
    lamn_d = nc.dram_tensor("lamn", [1, 2], f32, kind="ExternalInput")
    out_d = nc.dram_tensor("outp", [S, D], f16, kind="ExternalOutput")

    tqc_np, tqs_np, tkc_np, tks_np = _tables()
    mlo_np, mc_np = _masks()
    tqc_d = nc.inline_tensor(tqc_np, "tab_qc")
    tqs_d = nc.inline_tensor(tqs_np, "tab_qs")
    tkc_d = nc.inline_tensor(tkc_np, "tab_kc")
    tks_d = nc.inline_tensor(tks_np, "tab_ks")
    mlo_d = nc.inline_tensor(mlo_np, "mask_lo")
    mc_d = nc.inline_tensor(mc_np, "mask_c")
    idb_d = nc.inline_tensor(np.eye(128, dtype=BF), "ident_bf")
    ones_d = nc.inline_tensor(np.ones((128, 1), dtype=np.float32), "ones_f32")

    with tile.TileContext(nc) as tc:
        with tc.tile_pool(name="xpool", bufs=1) as xp, \
             tc.tile_pool(name="wpool", bufs=1) as wp, \
             tc.tile_pool(name="pers", bufs=1) as pers:

            # ---- input DMAs, critical-path first: interleave the weight
            # slices and first x chunk so every queue carries bytes the
            # first projection matmuls need.
            wq = wp.tile([128, NKT, 2, 128], bf)
            wk = wp.tile([128, NKT, 128], bf)
            wv = wp.tile([128, NKT, 128], bf)
            xt = xp.tile([128, NKT, S], bf)
            nc.sync.dma_start(out=wq[:, 0:1], in_=wq_d[:, 0:1])
            nc.sync.dma_start(out=wk[:, 0:2], in_=wk_d[:, 0:2])
            nc.sync.dma_start(out=wv[:, 0:2], in_=wv_d[:, 0:2])
            nc.sync.dma_start(out=xt[:, 0:1, 0:512], in_=xt_d[:, 0:1, 0:512])
            nc.sync.dma_start(out=xt[:, 1:2, 0:512], in_=xt_d[:, 1:2, 0:512])
            nc.sync.dma_start(out=wq[:, 1:3], in_=wq_d[:, 1:3])
            nc.sync.dma_start(out=xt[:, 2:3, 0:512], in_=xt_d[:, 2:3, 0:512])
            nc.sync.dma_start(out=xt[:, 3:4, 0:512], in_=xt_d[:, 3:4, 0:512])
            nc.sync.dma_start(out=wq[:, 3:6], in_=wq_d[:, 3:6])
            nc.sync.dma_start(out=wk[:, 2:8], in_=wk_d[:, 2:8])
            nc.sync.dma_start(out=wv[:, 2:8], in_=wv_d[:, 2:8])
            nc.sync.dma_start(out=xt[:, 4:6, 0:512], in_=xt_d[:, 4:6, 0:512])
            nc.sync.dma_start(out=xt[:, 6:8, 0:512], in_=xt_d[:, 6:8, 0:512])
            nc.sync.dma_start(out=wq[:, 6:11], in_=wq_d[:, 6:11])
            nc.sync.dma_start(out=wk[:, 8:16], in_=wk_d[:, 8:16])
            nc.sync.dma_start(out=wv[:, 8:16], in_=wv_d[:, 8:16])
            nc.sync.dma_start(out=xt[:, 8:12, 0:512], in_=xt_d[:, 8:12, 0:512])
            nc.sync.dma_start(out=wq[:, 11:16], in_=wq_d[:, 11:16])
            nc.sync.dma_start(out=xt[:, 12:16, 0:512], in_=xt_d[:, 12:16, 0:512])
            tqc = wp.tile([128, S], f16)
            tqs = wp.tile([128, S], f16)
            tkc = wp.tile([128, S], f16)
            tks = wp.tile([128, S], f16)
            nc.sync.dma_start(out=tqc[:], in_=tqc_d[:])
            nc.sync.dma_start(out=tqs[:], in_=tqs_d[:])
            nc.sync.dma_start(out=tkc[:], in_=tkc_d[:])
            nc.sync.dma_start(out=tks[:], in_=tks_d[:])
            mlo = wp.tile([128, 128], bf)
            nc.sync.dma_start(out=mlo[:], in_=mlo_d[:])
            mcs = wp.tile([128, 128], bf)
            nc.sync.dma_start(out=mcs[:], in_=mc_d[:])
            idb = wp.tile([128, 128], bf)
            nc.sync.dma_start(out=idb[:], in_=idb_d[:])
            lamn = wp.tile([1, 2], f32)
            nc.sync.dma_start(out=lamn[:], in_=lamn_d[:])
            ones = wp.tile([128, 1], f32)
            nc.sync.dma_start(out=ones[:], in_=ones_d[:])
            for ch in range(1, 4):
                sl = slice(512 * ch, 512 * (ch + 1))
                for kq in range(4):
                    ks = slice(4 * kq, 4 * kq + 4)
                    nc.sync.dma_start(out=xt[:, ks, sl], in_=xt_d[:, ks, sl])
            wo = wp.tile([128, 2, D], bf)
            nc.sync.dma_start(out=wo[:, :, 0:1024], in_=wo_d[:, :, 0:1024])
            nc.sync.dma_start(out=wo[:, :, 1024:2048], in_=wo_d[:, :, 1024:2048])
            lamb = wp.tile([128, 2], f32)
            nc.gpsimd.partition_broadcast(lamb[:], lamn[:])

            qt = pers.tile([128, 2, S], f16)      # RoPE'd scaled q, hd-major
            kt = pers.tile([128, S], f16)         # RoPE'd k, hd-major
            vsm = pers.tile([128, NQT, 128], bf)  # v, S-major [s, hd]
            att = pers.tile([128, 2, S], bf)      # attention out^T, hd-major

            # ---- Phase 1: projections + RoPE + v transpose ----
            with tc.tile_pool(name="pp", bufs=1, space="PSUM") as pp, \
                 tc.tile_pool(name="pt", bufs=2) as pt:
                for nch in range(4):
                    sl = slice(nch * 512, (nch + 1) * 512)
                    ps_q0 = pp.tile([128, 512], f32, tag="pq0", bufs=2)
                    ps_q1 = pp.tile([128, 512], f32, tag="pq1", bufs=2)
                    ps_k = pp.tile([128, 512], f32, tag="pk", bufs=1)
                    ps_v = pp.tile([128, 512], f32, tag="pv", bufs=1)
                    for kti in range(NKT):
                        st = kti == 0
                        sp = kti == NKT - 1
                        rhs = xt[:, kti, sl]
                        nc.tensor.matmul(ps_q0[:], wq[:, kti, 0, :], rhs, start=st, stop=sp)
                        nc.tensor.matmul(ps_q1[:], wq[:, kti, 1, :], rhs, start=st, stop=sp)
                        nc.tensor.matmul(ps_k[:], wk[:, kti, :], rhs, start=st, stop=sp)
                        nc.tensor.matmul(ps_v[:], wv[:, kti, :], rhs, start=st, stop=sp)
                    for ps, outt, tabc, tabs in (
                            (ps_q0, qt[:, 0, sl], tqc, tqs),
                            (ps_q1, qt[:, 1, sl], tqc, tqs),
                            (ps_k, kt[:, sl], tkc, tks)):
                        f = pt.tile([128, 512], f16, tag="f", bufs=4)
                        nc.scalar.copy(out=f[:], in_=ps[:])
                        m2 = pt.tile([128, 512], f16, tag="m2")
                        # rotate_half partner * sin
                        nc.vector.tensor_mul(m2[0:64, :], f[64:128, :], tabs[64:128, sl])
                        nc.vector.tensor_mul(m2[64:128, :], f[0:64, :], tabs[0:64, sl])
                        m1 = pt.tile([128, 512], f16, tag="m1")
                        nc.vector.tensor_mul(m1[:], f[:], tabc[:, sl])
                        nc.vector.tensor_sub(outt[0:64, :], m1[0:64, :], m2[0:64, :])
                        nc.vector.tensor_add(outt[64:128, :], m1[64:128, :], m2[64:128, :])
                    vtmp = pt.tile([128, 512], bf, tag="vtmp")
                    nc.scalar.copy(out=vtmp[:], in_=ps_v[:])
                    ps_tv = pp.tile([128, 4, 128], bf, tag="ptv", bufs=2)
                    for j in range(4):
                        nc.tensor.transpose(ps_tv[:, j, :], vtmp[:, 128 * j:128 * (j + 1)], idb[:])
                    nc.vector.tensor_copy(out=vsm[:, 4 * nch:4 * (nch + 1), :], in_=ps_tv[:])

            # ---- Phase 2+3: attention with 2-deep pipeline + Wo ----
            with tc.tile_pool(name="psc", bufs=1, space="PSUM") as psc, \
                 tc.tile_pool(name="pse", bufs=1) as pse, \
                 tc.tile_pool(name="psm", bufs=1) as psm:

                gn_t = {}

                def emit_scores(qi):
                    qsl = slice(qi * 128, (qi + 1) * 128)
                    kw = min(qi + 1, 5)
                    w = kw * 128
                    kstart = max(0, qi - 4)
                    for h in range(2):
                        ps_s = [psc.tile([128, WMAX], f32, tag="s", bufs=2,
                                         name=f"ps_s{half}")
                                for half in range(2)]
                        for half, ps in enumerate(ps_s):
                            hp = slice(64 * half, 64 * half + 64)
                            lhs = qt[hp, h, qsl]
                            ktw = kt[hp, kstart * 128:kstart * 128 + w]
                            if qi >= 4:
                                nc.tensor.matmul(ps[:, 0:128], idb[:], mlo[:],
                                                 start=True, stop=False)
                                nc.tensor.matmul(ps[:, 0:128], lhs, ktw[:, 0:128],
                                                 start=False, stop=True)
                                nc.tensor.matmul(ps[:, 128:512], lhs,
                                                 ktw[:, 128:512], start=True, stop=True)
                                nc.tensor.matmul(ps[:, 512:640], idb[:], mcs[:],
                                                 start=True, stop=False)
                                nc.tensor.matmul(ps[:, 512:640], lhs,
                                                 ktw[:, 512:640], start=False, stop=True)
                            else:
                                wu = (kw - 1) * 128
                                if wu > 0:
                                    nc.tensor.matmul(ps[:, 0:wu], lhs, ktw[:, 0:wu],
                                                     start=True, stop=True)
                                nc.tensor.matmul(ps[:, wu:w], idb[:], mcs[:],
                                                 start=True, stop=False)
                                nc.tensor.matmul(ps[:, wu:w], lhs, ktw[:, wu:w],
                                                 start=False, stop=True)

                        e1 = pse.tile([128, WMAX], bf, tag="e1", bufs=2)
                        e2 = pse.tile([128, WMAX], bf, tag="e2", bufs=2)
                        s1 = psm.tile([128, 1], f32, tag="s1", bufs=4)
                        s2 = psm.tile([128, 1], f32, tag="s2", bufs=4)
                        # e2 first so r2's reciprocal overlaps e1's exp
                        nc.scalar.activation(out=e2[:, 0:w], in_=ps_s[1][:, 0:w],
                                             func=EXP, accum_out=s2[:])
                        nc.scalar.activation(out=e1[:, 0:w], in_=ps_s[0][:, 0:w],
                                             func=EXP, accum_out=s1[:])
                        # cneg = -(lam * s1 / s2)   (lamn holds -lam)
                        r2 = psm.tile([128, 1], f32, tag="r2", bufs=4)
                        nc.vector.reciprocal(out=r2[:], in_=s2[:])
                        cneg = psm.tile([128, 1], f32, tag="cneg", bufs=4)
                        nc.gpsimd.tensor_scalar(
                            out=cneg[:], in0=s1[:], scalar1=lamb[:, h:h + 1],
                            scalar2=r2[:], op0=MULT, op1=MULT)
                        # g = relu(e1 + cneg*e2), normalized by its row sum
                        g0 = pse.tile([128, WMAX], bf, tag="g0", bufs=2)
                        nc.vector.scalar_tensor_tensor(
                            out=g0[:, 0:w], in0=e2[:, 0:w], scalar=cneg[:],
                            in1=e1[:, 0:w], op0=MULT, op1=ADD)
                        g = pse.tile([128, WMAX], bf, tag="g", bufs=2)
                        dsum = psm.tile([128, 1], f32, tag="dsum", bufs=4)
                        nc.vector.tensor_scalar(
                            out=g[:, 0:w], in0=g0[:, 0:w], scalar1=0.0,
                            scalar2=0.0, op0=MAX, op1=ADD, accum_out=dsum[:])
                        recd = psm.tile([128, 1], f32, tag="recd", bufs=4)
                        nc.vector.reciprocal(out=recd[:], in_=dsum[:])
                        gn = pse.tile([128, WMAX], bf, tag="gn", bufs=4)
                        nc.gpsimd.tensor_scalar(
                            out=gn[:, 0:w], in0=g[:, 0:w], scalar1=recd[:],
                            scalar2=0.0, op0=MULT, op1=ADD)
                        gn_t[(qi, h)] = gn

                def emit_ph3_pair(qj, pair):
                    qsl = slice(qj * 128, (qj + 1) * 128)
                    for dch in (2 * pair, 2 * pair + 1):
                        dsl = slice(dch * 512, (dch + 1) * 512)
                        ps_o = psc.tile([128, 512], f32, tag="o", bufs=2)
                        nc.tensor.matmul(ps_o[:], att[:, 0, qsl], wo[:, 0, dsl],
                                         start=True, stop=False)
                        nc.tensor.matmul(ps_o[:], att[:, 1, qsl], wo[:, 1, dsl],
                                         start=False, stop=True)
                        so = pse.tile([128, 512], f16, tag="so", bufs=4)
                        if dch % 2 == 0:
                            nc.scalar.copy(out=so[:], in_=ps_o[:])
                        else:
                            nc.vector.tensor_copy(out=so[:], in_=ps_o[:])
                        nc.sync.dma_start(out=out_d[qsl, dsl], in_=so[:])

                def emit_trav_ph3(qi, qj):
                    # trav(qi)'s two transpose groups interleaved with
                    # ph3(qj)'s matmul pairs so the PE never sits idle
                    # waiting for a gts/o-psum copy to release its buffer.
                    if qi is not None:
                        qsl = slice(qi * 128, (qi + 1) * 128)
                        kw = min(qi + 1, 5)
                        kstart = max(0, qi - 4)
                        ps_av = psc.tile([128, 2, 128], f32, tag="av", bufs=1)
                        gts2 = pse.tile([128, 5, 2, 128], bf, tag="gts", bufs=2)
                        for h in range(2):
                            gn = gn_t.pop((qi, h))
                            ps_tr = psc.tile([128, 5, 128], bf, tag="trg", bufs=1)
                            for j in range(kw):
                                nc.tensor.transpose(ps_tr[:, j, :],
                                                    gn[:, 128 * j:128 * (j + 1)], idb[:])
                            nc.vector.tensor_copy(out=gts2[:, 0:kw, h, :], in_=ps_tr[:, 0:kw, :])
                            if qj is not None:
                                emit_ph3_pair(qj, h)
                        for j in range(kw):
                            nc.tensor.matmul(ps_av[:, :, :], vsm[:, kstart + j, :],
                                             gts2[:, j, :, :],
                                             start=(j == 0), stop=(j == kw - 1))
                        nc.vector.tensor_copy(out=att[:, :, qsl], in_=ps_av[:])
                    elif qj is not None:
                        emit_ph3_pair(qj, 0)
                        emit_ph3_pair(qj, 1)

                for i in range(NQT + 2):
                    if i < NQT:
                        emit_scores(i)
                    tr = i - 1 if 0 <= i - 1 < NQT else None
                    p3 = i - 2 if 0 <= i - 2 < NQT else None
                    emit_trav_ph3(tr, p3)

    nc.compile()
    return nc


def get_program():
    if "nc" not in _CACHE:
        _CACHE["nc"] = _build_program()
    return _CACHE["nc"]


def _prep_inputs(x, Wq, Wk, Wv, Wo, lam):
    xt = np.ascontiguousarray(x.reshape(S, D).T.astype(BF)
                              .reshape(NKT, 128, S).transpose(1, 0, 2))
    in_maps = []
    for c in range(N_CORES):
        h0 = 2 * c
        kv = c // 2
        wq_c = np.ascontiguousarray(
            Wq[:, h0 * 128:(h0 + 2) * 128].astype(BF)
            .reshape(NKT, 128, 2, 128).transpose(1, 0, 2, 3))
        wk_c = np.ascontiguousarray(
            Wk[:, kv * 128:(kv + 1) * 128].astype(BF)
            .reshape(NKT, 128, 128).transpose(1, 0, 2))
        wv_c = np.ascontiguousarray(
            Wv[:, kv * 128:(kv + 1) * 128].astype(BF)
            .reshape(NKT, 128, 128).transpose(1, 0, 2))
        wo_c = np.ascontiguousarray(
            Wo[h0 * 128:(h0 + 2) * 128, :].astype(BF)
            .reshape(2, 128, D).transpose(1, 0, 2))
        lamn_c = np.array([[-float(lam[h0]), -float(lam[h0 + 1])]], dtype=np.float32)
        in_maps.append({"xt": xt, "wq": wq_c, "wk": wk_c, "wv": wv_c,
                        "wo": wo_c, "lamn": lamn_c})
    return in_maps


def kernel(x, Wq, Wk, Wv, Wo, lam):
    from concourse.bass_utils import run_bass_kernel_spmd

    nc = get_program()
    in_maps = _prep_inputs(np.asarray(x), np.asarray(Wq), np.asarray(Wk),
                           np.asarray(Wv), np.asarray(Wo), np.asarray(lam))
    res = run_bass_kernel_spmd(nc, in_maps, list(range(N_CORES)))
    out = np.zeros((S, D), dtype=np.float32)
    for c in range(N_CORES):
        out += res.results[c]["outp"].astype(np.float32)
    return out.reshape(1, S, D)


# revision 33
# speedup vs baseline: 1.0142x; 1.0055x over previous
"""Trainium2 Bass kernel for nn_ChimeraV2Block (dual-softmax differential
sliding-window attention block, B=1 S=2048 D=2048, 16 q-heads / 4 kv-heads,
head_dim 128, window 512).

Sharding: tensor-parallel over heads across 8 NeuronCores. Core c owns
q-heads {2c, 2c+1} and kv-head c//2 (GQA groups align with the split).
Wq/Wk/Wv column-sharded, Wo row-sharded; the 8 fp32 partial outputs are
summed on the host (the "all-reduce").

Schedule: phase 1 (projections + RoPE) is PE-bound; phase 2/3 (attention +
output projection) is software-pipelined two q-tiles deep so the per-tile
cross-engine chain (exp -> recip -> g0 -> relu -> gn) overlaps the next
tile's score matmuls, with the Wo matmuls and output DMA interleaved
per-tile. Masks are applied only to the two boundary 128-col subtiles of
each 640-wide score strip.
"""

import sys

if "/opt/trn_rl_repo" not in sys.path:
    sys.path.insert(0, "/opt/trn_rl_repo")

import numpy as np
import ml_dtypes

BF = ml_dtypes.bfloat16
F16 = np.float16

S = 2048
D = 2048
H = 16
HK = 4
HD = 128
WIN = 512
THETA = 10000.0
N_CORES = 8
NQT = S // 128          # 16 q row-tiles
NKT = D // 128          # 16 contraction tiles for the projections
WMAX = WIN + 128        # 640: max key-window width per q-tile
NEG = -1.0e30

_CACHE = {}


def _tables():
    """RoPE tables [128, S] fp16 with head-dim-duplicated frequencies
    (row p uses invf[p % 64]), so every op reads the table at the same
    base partition as its (possibly swapped) q operand. Q tables are
    pre-scaled by the attention scale 1/sqrt(64)."""
    invf = 1.0 / (THETA ** (np.arange(0, HD, 2, dtype=np.float64) / HD))  # [64]
    t = np.arange(S, dtype=np.float64)
    fr = np.outer(invf, t)  # [64, S]
    cosf = np.concatenate([np.cos(fr)] * 2, axis=0)
    sinf = np.concatenate([np.sin(fr)] * 2, axis=0)
    return (np.ascontiguousarray(cosf * 0.125, dtype=F16),
            np.ascontiguousarray(sinf * 0.125, dtype=F16),
            np.ascontiguousarray(cosf, dtype=F16),
            np.ascontiguousarray(sinf, dtype=F16))


def _masks():
    p = np.arange(128)[:, None]
    c = np.arange(128)[None, :]
    # interior first subtile: key col c (window start) allowed iff c >= p+1
    m_lo = np.where(c >= p + 1, 0.0, NEG).astype(BF)
    # causal subtile (diagonal block): allowed iff c <= p
    mc = np.where(c <= p, 0.0, NEG).astype(BF)
    return m_lo, mc


def _build_program():
    import concourse.bacc as bacc
    import concourse.tile as tile
    from concourse import mybir

    bf = mybir.dt.bfloat16
    f16 = mybir.dt.float16
    f32 = mybir.dt.float32
    EXP = mybir.ActivationFunctionType.Exp
    MULT = mybir.AluOpType.mult
    ADD = mybir.AluOpType.add
    MAX = mybir.AluOpType.max
    DIV = mybir.AluOpType.divide

    nc = bacc.Bacc("TRN2", target_bir_lowering=False, debug=False,
                   num_devices=N_CORES)

    xt_d = nc.dram_tensor("xt", [128, NKT, S], bf, kind="ExternalInput")
    wq_d = nc.dram_tensor("wq", [128, NKT, 2, 128], bf, kind="ExternalInput")
    wk_d = nc.dram_tensor("wk", [128, NKT, 128], bf, kind="ExternalInput")
    wv_d = nc.dram_tensor("wv", [128, NKT, 128], bf, kind="ExternalInput")
    wo_d = nc.dram_tensor("wo", [128, 2, D], bf, kind="ExternalInput")
    lamn_d = nc.dram_tensor("lamn", [1, 2], f32, kind="ExternalInput")
    out_d = nc.dram_tensor("outp", [S, D], f16, kind="ExternalOutput")

    tqc_np, tqs_np, tkc_np, tks_np = _tables()
    mlo_np, mc_np = _masks()
    tqc_d = nc.inline_tensor(tqc_np, "tab_qc")
    tqs_d = nc.inline_tensor(tqs_np, "tab_qs")
    tkc_d = nc.inline_tensor(tkc_np, "tab_kc")
    tks_d = nc.inline_tensor(tks_np, "tab_ks")
    mlo_d = nc.inline_tensor(mlo_np, "mask_lo")
    mc_d = nc.inline_tensor(mc_np, "mask_c")
    idb_d = nc.inline_tensor(np.eye(128, dtype=BF), "ident_bf")
    ones_d = nc.inline_tensor(np.ones((128, 1), dtype=np.float32), "ones_f32")

    with tile.TileContext(nc) as tc:
        with tc.tile_pool(name="xpool", bufs=1) as xp, \
             tc.tile_pool(name="wpool", bufs=1) as wp, \
             tc.tile_pool(name="pers", bufs=1) as pers:

            # ---- input DMAs, critical-path first: interleave the weight
            # slices and first x chunk so every queue carries bytes the
            # first projection matmuls need.
            wq = wp.tile([128, NKT, 2, 128], bf)
            wk = wp.tile([128, NKT, 128], bf)
            wv = wp.tile([128, NKT, 128], bf)
            xt = xp.tile([128, NKT, S], bf)
            nc.sync.dma_start(out=wq[:, 0:1], in_=wq_d[:, 0:1])
            nc.sync.dma_start(out=wk[:, 0:2], in_=wk_d[:, 0:2])
            nc.sync.dma_start(out=wv[:, 0:2], in_=wv_d[:, 0:2])
            nc.sync.dma_start(out=xt[:, 0:1, 0:512], in_=xt_d[:, 0:1, 0:512])
            nc.sync.dma_start(out=xt[:, 1:2, 0:512], in_=xt_d[:, 1:2, 0:512])
            nc.sync.dma_start(out=wq[:, 1:3], in_=wq_d[:, 1:3])
            nc.sync.dma_start(out=xt[:, 2:3, 0:512], in_=xt_d[:, 2:3, 0:512])
            nc.sync.dma_start(out=xt[:, 3:4, 0:512], in_=xt_d[:, 3:4, 0:512])
            nc.sync.dma_start(out=wq[:, 3:6], in_=wq_d[:, 3:6])
            nc.sync.dma_start(out=wk[:, 2:8], in_=wk_d[:, 2:8])
            nc.sync.dma_start(out=wv[:, 2:8], in_=wv_d[:, 2:8])
            nc.sync.dma_start(out=xt[:, 4:6, 0:512], in_=xt_d[:, 4:6, 0:512])
            nc.sync.dma_start(out=xt[:, 6:8, 0:512], in_=xt_d[:, 6:8, 0:512])
            nc.sync.dma_start(out=wq[:, 6:11], in_=wq_d[:, 6:11])
            nc.sync.dma_start(out=wk[:, 8:16], in_=wk_d[:, 8:16])
            nc.sync.dma_start(out=wv[:, 8:16], in_=wv_d[:, 8:16])
            nc.sync.dma_start(out=xt[:, 8:12, 0:512], in_=xt_d[:, 8:12, 0:512])
            nc.sync.dma_start(out=wq[:, 11:16], in_=wq_d[:, 11:16])
            nc.sync.dma_start(out=xt[:, 12:16, 0:512], in_=xt_d[:, 12:16, 0:512])
            tqc = wp.tile([128, S], f16)
            tqs = wp.tile([128, S], f16)
            tkc = wp.tile([128, S], f16)
            tks = wp.tile([128, S], f16)
            nc.sync.dma_start(out=tqc[:], in_=tqc_d[:])
            nc.sync.dma_start(out=tqs[:], in_=tqs_d[:])
            nc.sync.dma_start(out=tkc[:], in_=tkc_d[:])
            nc.sync.dma_start(out=tks[:], in_=tks_d[:])
            mlo = wp.tile([128, 128], bf)
            nc.sync.dma_start(out=mlo[:], in_=mlo_d[:])
            mcs = wp.tile([128, 128], bf)
            nc.sync.dma_start(out=mcs[:], in_=mc_d[:])
            idb = wp.tile([128, 128], bf)
            nc.sync.dma_start(out=idb[:], in_=idb_d[:])
            lamn = wp.tile([1, 2], f32)
            nc.sync.dma_start(out=lamn[:], in_=lamn_d[:])
            ones = wp.tile([128, 1], f32)
            nc.sync.dma_start(out=ones[:], in_=ones_d[:])
            for ch in range(1, 4):
                sl = slice(512 * ch, 512 * (ch + 1))
                for kq in range(4):
                    ks = slice(4 * kq, 4 * kq + 4)
                    nc.sync.dma_start(out=xt[:, ks, sl], in_=xt_d[:, ks, sl])
            wo = wp.tile([128, 2, D], bf)
            nc.sync.dma_start(out=wo[:, :, 0:1024], in_=wo_d[:, :, 0:1024])
            nc.sync.dma_start(out=wo[:, :, 1024:2048], in_=wo_d[:, :, 1024:2048])
            lamb = wp.tile([128, 2], f32)
            nc.gpsimd.partition_broadcast(lamb[:], lamn[:])

            qt = pers.tile([128, 2, S], f16)      # RoPE'd scaled q, hd-major
            kt = pers.tile([128, S], f16)         # RoPE'd k, hd-major
            vsm = pers.tile([128, NQT, 128], bf)  # v, S-major [s, hd]
            att = pers.tile([128, 2, S], bf)      # attention out^T, hd-major

            # ---- Phase 1: projections + RoPE + v transpose ----
            with tc.tile_pool(name="pp", bufs=1, space="PSUM") as pp, \
                 tc.tile_pool(name="pt", bufs=2) as pt:
                for nch in range(4):
                    sl = slice(nch * 512, (nch + 1) * 512)
                    ps_q0 = pp.tile([128, 512], f32, tag="pq0", bufs=2)
                    ps_q1 = pp.tile([128, 512], f32, tag="pq1", bufs=2)
                    ps_k = pp.tile([128, 512], f32, tag="pk", bufs=2)
                    ps_v = pp.tile([128, 512], f32, tag="pv", bufs=1)
                    for kti in range(NKT):
                        st = kti == 0
                        sp = kti == NKT - 1
                        rhs = xt[:, kti, sl]
                        nc.tensor.matmul(ps_q0[:], wq[:, kti, 0, :], rhs, start=st, stop=sp)
                        nc.tensor.matmul(ps_q1[:], wq[:, kti, 1, :], rhs, start=st, stop=sp)
                        nc.tensor.matmul(ps_k[:], wk[:, kti, :], rhs, start=st, stop=sp)
                        nc.tensor.matmul(ps_v[:], wv[:, kti, :], rhs, start=st, stop=sp)
                    vtmp = pt.tile([128, 512], bf, tag="vtmp")
                    nc.scalar.copy(out=vtmp[:], in_=ps_v[:])
                    for ps, outt, tabc, tabs in (
                            (ps_q0, qt[:, 0, sl], tqc, tqs),
                            (ps_q1, qt[:, 1, sl], tqc, tqs),
                            (ps_k, kt[:, sl], tkc, tks)):
                        f = pt.tile([128, 512], f16, tag="f", bufs=4)
                        nc.scalar.copy(out=f[:], in_=ps[:])
                        m2 = pt.tile([128, 512], f16, tag="m2")
                        # rotate_half partner * sin
                        nc.vector.tensor_mul(m2[0:64, :], f[64:128, :], tabs[64:128, sl])
                        nc.vector.tensor_mul(m2[64:128, :], f[0:64, :], tabs[0:64, sl])
                        m1 = pt.tile([128, 512], f16, tag="m1")
                        nc.vector.tensor_mul(m1[:], f[:], tabc[:, sl])
                        nc.vector.tensor_sub(outt[0:64, :], m1[0:64, :], m2[0:64, :])
                        nc.vector.tensor_add(outt[64:128, :], m1[64:128, :], m2[64:128, :])
                    ps_tv = pp.tile([128, 4, 128], bf, tag="ptv", bufs=1)
                    for j in range(4):
                        nc.tensor.transpose(ps_tv[:, j, :], vtmp[:, 128 * j:128 * (j + 1)], idb[:])
                    nc.vector.tensor_copy(out=vsm[:, 4 * nch:4 * (nch + 1), :], in_=ps_tv[:])

            # ---- Phase 2+3: attention with 2-deep pipeline + Wo ----
            with tc.tile_pool(name="psc", bufs=1, space="PSUM") as psc, \
                 tc.tile_pool(name="pse", bufs=1) as pse, \
                 tc.tile_pool(name="psm", bufs=1) as psm:

                gn_t = {}

                def emit_scores(qi):
                    qsl = slice(qi * 128, (qi + 1) * 128)
                    kw = min(qi + 1, 5)
                    w = kw * 128
                    kstart = max(0, qi - 4)
                    for h in range(2):
                        ps_s = [psc.tile([128, WMAX], f32, tag="s", bufs=2,
                                         name=f"ps_s{half}")
                                for half in range(2)]
                        for half, ps in enumerate(ps_s):
                            hp = slice(64 * half, 64 * half + 64)
                            lhs = qt[hp, h, qsl]
                            ktw = kt[hp, kstart * 128:kstart * 128 + w]
                            if qi >= 4:
                                nc.tensor.matmul(ps[:, 0:128], idb[:], mlo[:],
                                                 start=True, stop=False)
                                nc.tensor.matmul(ps[:, 0:128], lhs, ktw[:, 0:128],
                                                 start=False, stop=True)
                                nc.tensor.matmul(ps[:, 128:512], lhs,
                                                 ktw[:, 128:512], start=True, stop=True)
                                nc.tensor.matmul(ps[:, 512:640], idb[:], mcs[:],
                                                 start=True, stop=False)
                                nc.tensor.matmul(ps[:, 512:640], lhs,
                                                 ktw[:, 512:640], start=False, stop=True)
                            else:
                                wu = (kw - 1) * 128
                                if wu > 0:
                                    nc.tensor.matmul(ps[:, 0:wu], lhs, ktw[:, 0:wu],
                                                     start=True, stop=True)
                                nc.tensor.matmul(ps[:, wu:w], idb[:], mcs[:],
                                                 start=True, stop=False)
                                nc.tensor.matmul(ps[:, wu:w], lhs, ktw[:, wu:w],
                                                 start=False, stop=True)

                        e1 = pse.tile([128, WMAX], bf, tag="e1", bufs=2)
                        e2 = pse.tile([128, WMAX], bf, tag="e2", bufs=2)
                        s1 = psm.tile([128, 1], f32, tag="s1", bufs=4)
                        s2 = psm.tile([128, 1], f32, tag="s2", bufs=4)
                        # e2 first so r2's reciprocal overlaps e1's exp
                        nc.scalar.activation(out=e2[:, 0:w], in_=ps_s[1][:, 0:w],
                                             func=EXP, accum_out=s2[:])
                        nc.scalar.activation(out=e1[:, 0:w], in_=ps_s[0][:, 0:w],
                                             func=EXP, accum_out=s1[:])
                        # cneg = -(lam * s1 / s2)   (lamn holds -lam)
                        r2 = psm.tile([128, 1], f32, tag="r2", bufs=4)
                        nc.vector.reciprocal(out=r2[:], in_=s2[:])
                        cneg = psm.tile([128, 1], f32, tag="cneg", bufs=4)
                        nc.gpsimd.tensor_scalar(
                            out=cneg[:], in0=s1[:], scalar1=lamb[:, h:h + 1],
                            scalar2=r2[:], op0=MULT, op1=MULT)
                        # g = relu(e1 + cneg*e2), normalized by its row sum
                        g0 = pse.tile([128, WMAX], bf, tag="g0", bufs=2)
                        nc.vector.scalar_tensor_tensor(
                            out=g0[:, 0:w], in0=e2[:, 0:w], scalar=cneg[:],
                            in1=e1[:, 0:w], op0=MULT, op1=ADD)
                        g = pse.tile([128, WMAX], bf, tag="g", bufs=2)
                        dsum = psm.tile([128, 1], f32, tag="dsum", bufs=4)
                        nc.vector.tensor_scalar(
                            out=g[:, 0:w], in0=g0[:, 0:w], scalar1=0.0,
                            scalar2=0.0, op0=MAX, op1=ADD, accum_out=dsum[:])
                        recd = psm.tile([128, 1], f32, tag="recd", bufs=4)
                        nc.vector.reciprocal(out=recd[:], in_=dsum[:])
                        gn = pse.tile([128, WMAX], bf, tag="gn", bufs=4)
                        nc.gpsimd.tensor_scalar(
                            out=gn[:, 0:w], in0=g[:, 0:w], scalar1=recd[:],
                            scalar2=0.0, op0=MULT, op1=ADD)
                        gn_t[(qi, h)] = gn

                def emit_ph3_pair(qj, pair):
                    qsl = slice(qj * 128, (qj + 1) * 128)
                    for dch in (2 * pair, 2 * pair + 1):
                        dsl = slice(dch * 512, (dch + 1) * 512)
                        ps_o = psc.tile([128, 512], f32, tag="o", bufs=2)
                        nc.tensor.matmul(ps_o[:], att[:, 0, qsl], wo[:, 0, dsl],
                                         start=True, stop=False)
                        nc.tensor.matmul(ps_o[:], att[:, 1, qsl], wo[:, 1, dsl],
                                         start=False, stop=True)
                        so = pse.tile([128, 512], f16, tag="so", bufs=4)
                        if dch % 2 == 0:
                            nc.scalar.copy(out=so[:], in_=ps_o[:])
                        else:
                            nc.vector.tensor_copy(out=so[:], in_=ps_o[:])
                        nc.sync.dma_start(out=out_d[qsl, dsl], in_=so[:])

                def emit_trav_ph3(qi, qj):
                    # trav(qi)'s two transpose groups interleaved with
                    # ph3(qj)'s matmul pairs so the PE never sits idle
                    # waiting for a gts/o-psum copy to release its buffer.
                    if qi is not None:
                        qsl = slice(qi * 128, (qi + 1) * 128)
                        kw = min(qi + 1, 5)
                        kstart = max(0, qi - 4)
                        ps_av = psc.tile([128, 2, 128], f32, tag="av", bufs=1)
                        gts2 = pse.tile([128, 5, 2, 128], bf, tag="gts", bufs=2)
                        for h in range(2):
                            gn = gn_t.pop((qi, h))
                            ps_tr = psc.tile([128, 5, 128], bf, tag="trg", bufs=1)
                            for j in range(kw):
                                nc.tensor.transpose(ps_tr[:, j, :],
                                                    gn[:, 128 * j:128 * (j + 1)], idb[:])
                            if h == 0:
                                nc.scalar.copy(out=gts2[:, 0:kw, h, :], in_=ps_tr[:, 0:kw, :])
                            else:
                                nc.vector.tensor_copy(out=gts2[:, 0:kw, h, :], in_=ps_tr[:, 0:kw, :])
                            if qj is not None:
                                emit_ph3_pair(qj, h)
                        for j in range(kw):
                            nc.tensor.matmul(ps_av[:, :, :], vsm[:, kstart + j, :],
                                             gts2[:, j, :, :],
                                             start=(j == 0), stop=(j == kw - 1))
                        nc.vector.tensor_copy(out=att[:, :, qsl], in_=ps_av[:])
                    elif qj is not None:
                        emit_ph3_pair(qj, 0)
                        emit_ph3_pair(qj, 1)

                for i in range(NQT + 2):
                    if i < NQT:
                        emit_scores(i)
                    tr = i - 1 if 0 <= i - 1 < NQT else None
                    p3 = i - 2 if 0 <= i - 2 < NQT else None
                    emit_trav_ph3(tr, p3)

    nc.compile()
    return nc


def get_program():
    if "nc" not in _CACHE:
        _CACHE["nc"] = _build_program()
    return _CACHE["nc"]


def _prep_inputs(x, Wq, Wk, Wv, Wo, lam):
    xt = np.ascontiguousarray(x.reshape(S, D).T.astype(BF)
                              .reshape(NKT, 128, S).transpose(1, 0, 2))
    in_maps = []
    for c in range(N_CORES):
        h0 = 2 * c
        kv = c // 2
        wq_c = np.ascontiguousarray(
            Wq[:, h0 * 128:(h0 + 2) * 128].astype(BF)
            .reshape(NKT, 128, 2, 128).transpose(1, 0, 2, 3))
        wk_c = np.ascontiguousarray(
            Wk[:, kv * 128:(kv + 1) * 128].astype(BF)
            .reshape(NKT, 128, 128).transpose(1, 0, 2))
        wv_c = np.ascontiguousarray(
            Wv[:, kv * 128:(kv + 1) * 128].astype(BF)
            .reshape(NKT, 128, 128).transpose(1, 0, 2))
        wo_c = np.ascontiguousarray(
            Wo[h0 * 128:(h0 + 2) * 128, :].astype(BF)
            .reshape(2, 128, D).transpose(1, 0, 2))
        lamn_c = np.array([[-float(lam[h0]), -float(lam[h0 + 1])]], dtype=np.float32)
        in_maps.append({"xt": xt, "wq": wq_c, "wk": wk_c, "wv": wv_c,
                        "wo": wo_c, "lamn": lamn_c})
    return in_maps


def kernel(x, Wq, Wk, Wv, Wo, lam):
    from concourse.bass_utils import run_bass_kernel_spmd

    nc = get_program()
    in_maps = _prep_inputs(np.asarray(x), np.asarray(Wq), np.asarray(Wk),
                           np.asarray(Wv), np.asarray(Wo), np.asarray(lam))
    res = run_bass_kernel_spmd(nc, in_maps, list(range(N_CORES)))
    out = np.zeros((S, D), dtype=np.float32)
    for c in range(N_CORES):
        out += res.results[c]["outp"].astype(np.float32)
    return out.reshape(1, S, D)
